# revision 18
# baseline (speedup 1.0000x reference)
"""Trainium2 Bass kernel for nn_CrossAttention (b=8, c=128, hw=4096, dim=64).

Sharding: data-parallel over batch — one batch element per NeuronCore (8 cores).

Per-core algorithm (channel-major [c, t] layout, t = h*w = 4096 tokens):
  - LayerNorm over channels is folded algebraically:
      G = W' @ x + (-colsum(W')) (x) mu     (rank-1 mean-subtract fused into
                                             the PE accumulation, K=1 matmul)
      proj = relu(G * r_bcast + b')
    where W' = W * ln_w and b' = W @ ln_b + b are host-folded, mu/r are the
    per-token channel stats, r = rsqrt(var+eps) = exp(-0.5*ln(var+eps)).
  - Channel-dim stats via ones-vector matmuls on PE (M=1 rows landing on
    psum partitions 0..3 of one [4, 512] bank), DMA'd straight from PSUM
    into [128, 16]-per-half stat tiles for lane-parallel stat math.
  - Fully pipelined startup: input DMA -> per-512-block stats (squares split
    gpsimd/vector) -> per-1024-block projections + v transposes, emitted so
    the PE queue never waits on a full prior phase.
  - Attention with transposed scores: sT[tj, ti] = k_blk.T @ q so softmax's
    exp applies per strip and pT feeds the A@V matmul with no transposes.
    No max-subtraction (scores are bounded); a constant shift cancels in the
    normalization. Softmax denominator via a fused ones-column in V (M=65).
  - 4 ti passes of 1024; each pass's epilogue (pav->sbuf copy, l gather,
    reciprocal, broadcast, output projection, normalize, DMA out) is emitted
    inside the next pass's jp loop so it overlaps with attention compute.
All matmuls use float32r (full-rate fp32 on the PE at N=512) except the
attention pair matmuls (bf16).
"""

import sys

if "/opt/trn_rl_repo" not in sys.path:
    sys.path.insert(0, "/opt/trn_rl_repo")

import numpy as np

B = 8
C = 128  # channels (x_dim == ctx_dim)
D = 64  # attention dim
T = 4096  # tokens = 64*64
EPS = 1e-5
SCALE = float(D) ** -0.5
SHIFT = 2.0  # constant subtracted inside exp; cancels in softmax normalization

_CACHE = {}


def _build_program():
    import contextlib

    import concourse.bass as bass
    import concourse.bacc as bacc
    import concourse.mybir as mybir
    import concourse.tile as tile

    f32 = mybir.dt.float32
    f32r = mybir.dt.float32r
    bf16 = mybir.dt.bfloat16
    FT = mybir.ActivationFunctionType
    OP = mybir.AluOpType

    nc = bacc.Bacc("TRN2", target_bir_lowering=False, debug=False, num_devices=B)

    x_d = nc.dram_tensor("x", [C, T], f32r, kind="ExternalInput")
    c_d = nc.dram_tensor("ctx", [C, T], f32r, kind="ExternalInput")
    wq_d = nc.dram_tensor("wq", [C, D], f32r, kind="ExternalInput")  # (Wq*ln_w).T
    wkv_d = nc.dram_tensor("wkv", [C, 2 * D], f32r, kind="ExternalInput")
    sq_d = nc.dram_tensor("sq", [1, D], f32r, kind="ExternalInput")  # -colsum
    skv_d = nc.dram_tensor("skv", [1, 2 * D], f32r, kind="ExternalInput")
    bq_d = nc.dram_tensor("bq", [D, 1], f32, kind="ExternalInput")
    bkv_d = nc.dram_tensor("bkv", [2 * D, 1], f32, kind="ExternalInput")
    wo_d = nc.dram_tensor("wo", [D + 1, C], f32r, kind="ExternalInput")  # [Wout.T; bout]
    id_d = nc.dram_tensor("ident", [D, D], f32, kind="ExternalInput")
    out_d = nc.dram_tensor("out", [C, T], f32, kind="ExternalOutput")
    rx_scr = nc.dram_tensor("rx_scr", [T], f32r)
    rc_scr = nc.dram_tensor("rc_scr", [T], f32r)
    rl_scr = nc.dram_tensor("rl_scr", [T], f32r)

    NJ = T // 128  # 32 key strips
    NPASS = 4
    SPAN = T // NPASS  # 1024 ti per pass
    NS = T // 128  # 32 cols in the [128, NS] stat reshape

    with (
        tile.TileContext(nc) as tc,
        nc.allow_low_precision(
            reason="float32r tensors feed full-rate PE matmuls; values are "
            "fp32-resident and only rounded inside the PE"
        ),
    ):
        with contextlib.ExitStack() as ctx:
            const = ctx.enter_context(tc.tile_pool(name="const", bufs=1))
            big = ctx.enter_context(tc.tile_pool(name="big", bufs=1))
            st32 = ctx.enter_context(tc.tile_pool(name="st32", bufs=1))
            sqp = ctx.enter_context(tc.tile_pool(name="sqp", bufs=4))
            prep = ctx.enter_context(tc.tile_pool(name="prep", bufs=2))
            bcp = ctx.enter_context(tc.tile_pool(name="bcp", bufs=2))
            stgp = ctx.enter_context(tc.tile_pool(name="stgp", bufs=2))
            ptp = ctx.enter_context(tc.tile_pool(name="ptp", bufs=4))
            outp = ctx.enter_context(tc.tile_pool(name="outp", bufs=2))
            ltp = ctx.enter_context(tc.tile_pool(name="ltp", bufs=2))

            # ---- constants ----
            wq_sb = const.tile([C, D], f32r)
            wkv_sb = const.tile([C, 2 * D], f32r)
            sq_sb = const.tile([1, D], f32r)
            skv_sb = const.tile([1, 2 * D], f32r)
            bq_sb = const.tile([D, 1], f32)
            bkv_sb = const.tile([2 * D, 1], f32)
            wo_sb = const.tile([D + 1, C], f32r)
            id_sb = const.tile([C, D], bf16)
            ones_sb = const.tile([C, 32], f32r)
            eps_sb = const.tile([C, 1], f32)
            shift_sb = const.tile([C, 1], f32)
            nc.sync.dma_start(wq_sb[:], wq_d.ap())
            nc.sync.dma_start(wkv_sb[:], wkv_d.ap())
            nc.sync.dma_start(sq_sb[:], sq_d.ap())
            nc.sync.dma_start(skv_sb[:], skv_d.ap())
            nc.sync.dma_start(bq_sb[:], bq_d.ap())
            nc.sync.dma_start(bkv_sb[:], bkv_d.ap())
            nc.sync.dma_start(wo_sb[:], wo_d.ap())
            # identity needed at partitions 64..127 (v lives there in kv_sb)
            nc.gpsimd.dma_start(id_sb[D : 2 * D, :], id_d.ap())
            nc.vector.memset(ones_sb[:].bitcast(f32), 1.0)
            nc.vector.memset(eps_sb[:], EPS)
            nc.vector.memset(shift_sb[:], -SHIFT)

            # ---- big persistent tensors ----
            x_sb = big.tile([C, T], f32r)
            c_sb = big.tile([C, T], f32r)
            q2 = big.tile([128, T], bf16)
            kv_sb = big.tile([2 * D, T], bf16)
            k2 = big.tile([128, T], bf16)
            # one tile per key strip so AV's dependency is per-strip, not on
            # the whole v array
            v_tok = [
                big.tile([128, D + 1], bf16, name=f"v_tok{j}") for j in range(NJ)
            ]
            attn_sb = big.tile([D + 1, T], f32r)

            # input DMA split 4x per 512-block so the first block lands in
            # ~3us instead of ~12us (one queue per 128-col slice)
            for n in range(8):
                for f in range(4):
                    sl = slice(n * 512 + f * 128, n * 512 + (f + 1) * 128)
                    nc.sync.dma_start(x_sb[:, sl], x_d.ap()[:, sl])
                    nc.sync.dma_start(c_sb[:, sl], c_d.ap()[:, sl])

            # v' ones column: col D = 1.0; transposes fill cols 0:D
            for j in range(NJ):
                nc.vector.memset(v_tok[j][:, D : D + 1], 1.0)

            # stat tiles: [128, NS] layout, element (p, 4n+i) = token 512n+4p+i
            xs_t = st32.tile([128, NS], f32r)
            xss_t = st32.tile([128, NS], f32r)
            cs_t = st32.tile([128, NS], f32r)
            css_t = st32.tile([128, NS], f32r)
            mux_t = st32.tile([128, NS], f32r)
            rx_t = st32.tile([128, NS], f32r)
            muc_t = st32.tile([128, NS], f32r)
            rc_t = st32.tile([128, NS], f32r)

            actx = contextlib.ExitStack()
            pstp = actx.enter_context(
                tc.tile_pool(name="pst", bufs=1, space="PSUM")
            )
            pprp = actx.enter_context(
                tc.tile_pool(name="ppr", bufs=2, space="PSUM")
            )
            ptrp = actx.enter_context(
                tc.tile_pool(name="ptr", bufs=2, space="PSUM")
            )

            def stats_block(n):
                sl = slice(n * 512, (n + 1) * 512)
                xsq = sqp.tile([C, 512], f32r, tag="sq")
                csq = sqp.tile([C, 512], f32r, tag="sqc")
                nc.gpsimd.tensor_mul(xsq[:], x_sb[:, sl], x_sb[:, sl])
                nc.vector.tensor_mul(csq[:], c_sb[:, sl], c_sb[:, sl])
                for half, (r0, r1, d0, d1) in enumerate(
                    (
                        (x_sb[:, sl], xsq[:], xs_t, xss_t),
                        (c_sb[:, sl], csq[:], cs_t, css_t),
                    )
                ):
                    pst = pstp.tile([32, 1024], f32, tag=f"pst{half}")
                    nc.tensor.matmul(pst[:, 0:512], ones_sb[:], r0)
                    nc.tensor.matmul(pst[:, 512:1024], ones_sb[:], r1)
                    # [1, 1024] psum row -> sbuf (ACT for x, DVE for ctx so
                    # neither engine serializes stats) -> [128, 4] reshape
                    # DMAs: token 512n + 4p + i
                    row = sqp.tile([1, 1024], f32r, tag=f"strow{half}")
                    if half == 0:
                        nc.scalar.activation(row[:], pst[0:1, :], FT.Copy)
                    else:
                        nc.vector.tensor_copy(row[:], pst[0:1, :])
                    nc.sync.dma_start(d0[:, 4 * n : 4 * n + 4], row[0:1, 0:512])
                    nc.sync.dma_start(d1[:, 4 * n : 4 * n + 4], row[0:1, 512:1024])

            scr_ap = lambda h: h.ap().rearrange("(c p i) -> p c i", c=8, p=128, i=4)

            def stats_math_half(h):
                cs = slice(16 * h, 16 * h + 16)
                for s_t, ss_t, mu_t, r_t, scr in (
                    (xs_t, xss_t, mux_t, rx_t, rx_scr),
                    (cs_t, css_t, muc_t, rc_t, rc_scr),
                ):
                    mu2_t = st32.tile([128, 16], f32r, tag="mu2")
                    var_t = st32.tile([128, 16], f32r, tag="var")
                    nc.vector.tensor_scalar_mul(mu_t[:, cs], s_t[:, cs], 1.0 / C)
                    nc.vector.tensor_mul(mu2_t[:], mu_t[:, cs], mu_t[:, cs])
                    nc.vector.scalar_tensor_tensor(
                        var_t[:], ss_t[:, cs], 1.0 / C, mu2_t[:], OP.mult, OP.subtract
                    )
                    # r = 1/sqrt(var+eps); Sqrt shares an ACT table set with
                    # Copy so stats cause no table churn (Exp loads once for
                    # attention)
                    nc.scalar.activation(var_t[:], var_t[:], FT.Sqrt, bias=eps_sb[:])
                    nc.vector.reciprocal(r_t[:, cs], var_t[:])
                    nc.sync.dma_start(scr_ap(scr)[:, 4 * h : 4 * h + 4, :], r_t[:, cs])

            # projections: G = W' @ x - s (x) mu ; out = relu(G*r + b)
            # per-512 granularity; psum mul on DVE, bias+relu on gpsimd
            def project(w_sb, s_sb, b_sb, src_sb, mu_t, r_scr, dst_sb, m, n):
                for g in range(2):
                    n5 = 2 * n + g
                    mu_stg = stgp.tile([1, 512], f32r, tag="mustg")
                    nc.sync.dma_start(
                        mu_stg[0:1, :], mu_t[:, 4 * n5 : 4 * n5 + 4]
                    )
                    rbc = bcp.tile([128, 512], f32r, tag="rbc")
                    nc.sync.dma_start(
                        rbc[0:m, :],
                        bass.AP(r_scr, n5 * 512, [[0, m], [1, 512]]),
                    )
                    sl = slice(n5 * 512, (n5 + 1) * 512)
                    ps = pprp.tile([128, 512], f32, tag="pp")
                    po = ps[0:m, :]
                    nc.tensor.matmul(
                        po, w_sb[:], src_sb[:, sl], start=True, stop=False
                    )
                    nc.tensor.matmul(
                        po, s_sb[:], mu_stg[0:1, :], start=False, stop=True
                    )
                    pre = prep.tile([128, 512], f32, tag="pre")
                    nc.vector.tensor_mul(pre[0:m, :], po, rbc[0:m, :])
                    nc.vector.tensor_scalar(
                        dst_sb[0:m, sl], pre[0:m, :], b_sb[:], 0.0,
                        op0=OP.add, op1=OP.max,
                    )

            def proj_block(n):
                project(wkv_sb, skv_sb, bkv_sb, c_sb, muc_t, rc_scr, kv_sb, 2 * D, n)
                project(wq_sb, sq_sb, bq_sb, x_sb, mux_t, rx_scr, q2, D, n)
                sl = slice(n * 1024, (n + 1) * 1024)
                # v (kv rows D..2D) -> token-major tiles [tj, d]
                for j in range(8 * n, 8 * n + 8):
                    tp = ptrp.tile([128, D], bf16)
                    nc.tensor.matmul(
                        tp[:],
                        kv_sb[D : 2 * D, j * 128 : (j + 1) * 128],
                        id_sb[D : 2 * D, :],
                        is_transpose=True,
                    )
                    nc.vector.tensor_copy(v_tok[j][:, 0:D], tp[:])
                # duplicate q and k into both partition halves for row-packed
                # sim pairs (row group 64-127 streams from partitions 64-127)
                nc.sync.dma_start(q2[D:128, sl], q2[0:D, sl])
                nc.sync.dma_start(k2[0:D, sl], kv_sb[0:D, sl])
                nc.sync.dma_start(k2[D:128, sl], kv_sb[0:D, sl])

            # ---- phase A: pipelined stats + projections ----
            for n in range(4):
                stats_block(n)
            stats_math_half(0)
            for n in range(4, 8):
                stats_block(n)
            proj_block(0)
            proj_block(1)
            stats_math_half(1)
            proj_block(2)
            proj_block(3)

            actx.close()

            # ---- phase B: attention with interleaved per-pass epilogue ----
            pssp = ctx.enter_context(
                tc.tile_pool(name="pss", bufs=2, space="PSUM")
            )
            pavp = ctx.enter_context(
                tc.tile_pool(name="pav", bufs=1, space="PSUM")
            )
            poutp = ctx.enter_context(
                tc.tile_pool(name="pout", bufs=1, space="PSUM")
            )

            def tail(p2):
                # epilogue for pass p2 (ti strip p2*SPAN .. +SPAN); attn strip
                # already copied out of psum at end of pass p2
                sl4 = slice(p2 * SPAN, (p2 + 1) * SPAN)
                lt = ltp.tile([128, 8], f32r, tag="lt")
                rlt = ltp.tile([128, 8], f32r, tag="rlt")
                for c2 in range(2):
                    nc.sync.dma_start(
                        lt[:, c2 * 4 : (c2 + 1) * 4],
                        attn_sb[D : D + 1, p2 * SPAN + c2 * 512 : p2 * SPAN + (c2 + 1) * 512],
                    )
                nc.vector.reciprocal(rlt[:], lt[:])
                nc.sync.dma_start(
                    scr_ap(rl_scr)[:, 2 * p2 : 2 * p2 + 2, :], rlt[:]
                )
                rlbc = bcp.tile([128, 1024], f32r, tag="rbc")
                nc.sync.dma_start(
                    rlbc[:], bass.AP(rl_scr, p2 * SPAN, [[0, C], [1, 1024]])
                )
                po = poutp.tile([C, 1024], f32)
                for g in range(2):
                    sl = slice(p2 * SPAN + g * 512, p2 * SPAN + (g + 1) * 512)
                    nc.tensor.matmul(
                        po[:, g * 512 : (g + 1) * 512], wo_sb[:], attn_sb[:, sl]
                    )
                ot = outp.tile([C, 1024], f32)
                nc.vector.tensor_mul(ot[:], po[:], rlbc[:])
                nc.sync.dma_start(out_d.ap()[:, sl4], ot[:])

            for p2 in range(NPASS):
                pav = pavp.tile([D + 1, SPAN], f32)
                for jp in range(NJ // 2):
                    if p2 > 0 and jp == 2:
                        tail(p2 - 1)
                    jA, jB = 2 * jp, 2 * jp + 1
                    kA = k2[0:D, jA * 128 : (jA + 1) * 128]
                    kB = k2[D:128, jB * 128 : (jB + 1) * 128]
                    vA = v_tok[jA][:]
                    vB = v_tok[jB][:]
                    for c in range(2):
                        ti0 = p2 * SPAN + c * 512
                        pss = pssp.tile([128, 1024], f32)
                        nc.tensor.matmul(
                            pss[:, 0:512], kA, q2[0:D, ti0 : ti0 + 512]
                        )
                        nc.tensor.matmul(
                            pss[:, 512:1024], kB, q2[D:128, ti0 : ti0 + 512]
                        )
                        pt = ptp.tile([128, 1024], bf16, tag="pt")
                        nc.scalar.activation(
                            pt[:], pss[:], FT.Exp, bias=shift_sb[:], scale=SCALE
                        )
                        co = c * 512
                        nc.tensor.matmul(
                            pav[:, co : co + 512],
                            vA,
                            pt[:, 0:512],
                            start=(jp == 0),
                            stop=False,
                        )
                        nc.tensor.matmul(
                            pav[:, co : co + 512],
                            vB,
                            pt[:, 512:1024],
                            start=False,
                            stop=(jp == NJ // 2 - 1),
                        )
                # move pass result out of psum now so pav can recycle next pass
                nc.vector.tensor_copy(
                    attn_sb[:, p2 * SPAN : (p2 + 1) * SPAN], pav[:]
                )
            tail(NPASS - 1)

    nc.compile()
    return nc


def _get_program():
    if "nc" not in _CACHE:
        _CACHE["nc"] = _build_program()
    return _CACHE["nc"]


def _fold_weights(ln_x_w, ln_x_b, ln_c_w, ln_c_b, Wq, bq, Wkv, bkv, Wout, bout):
    f = np.float64
    Wq = np.asarray(Wq, f)
    Wkv = np.asarray(Wkv, f)
    Wout = np.asarray(Wout, f)
    wq_p = Wq * np.asarray(ln_x_w, f)[None, :]  # [D, C]
    wkv_p = Wkv * np.asarray(ln_c_w, f)[None, :]  # [2D, C]
    bq_p = Wq @ np.asarray(ln_x_b, f) + np.asarray(bq, f)
    bkv_p = Wkv @ np.asarray(ln_c_b, f) + np.asarray(bkv, f)
    wo_aug = np.concatenate([Wout.T, np.asarray(bout, f)[None, :]], axis=0)  # [D+1, C]
    return {
        "wq": np.ascontiguousarray(wq_p.T, np.float32),
        "wkv": np.ascontiguousarray(wkv_p.T, np.float32),
        "sq": np.ascontiguousarray(-wq_p.sum(axis=1)[None, :], np.float32),
        "skv": np.ascontiguousarray(-wkv_p.sum(axis=1)[None, :], np.float32),
        "bq": np.ascontiguousarray(bq_p[:, None], np.float32),
        "bkv": np.ascontiguousarray(bkv_p[:, None], np.float32),
        "wo": np.ascontiguousarray(wo_aug, np.float32),
        "ident": np.eye(D, dtype=np.float32),
    }


def _run(inputs, trace=False):
    from concourse.bass_utils import run_bass_kernel_spmd

    nc = _get_program()
    x = np.asarray(inputs["x"], np.float32)
    ctx = np.asarray(inputs["context"], np.float32)
    w = _fold_weights(
        inputs["ln_x_w"], inputs["ln_x_b"], inputs["ln_c_w"], inputs["ln_c_b"],
        inputs["Wq"], inputs["bq"], inputs["Wkv"], inputs["bkv"],
        inputs["Wout"], inputs["bout"],
    )
    in_maps = []
    for i in range(B):
        m = dict(w)
        m["x"] = np.ascontiguousarray(x[i].reshape(C, T))
        m["ctx"] = np.ascontiguousarray(ctx[i].reshape(C, T))
        in_maps.append(m)
    res = run_bass_kernel_spmd(nc, in_maps, list(range(B)), trace=trace)
    h = int(np.sqrt(T))
    out = np.stack([res.results[i]["out"].reshape(C, h, h) for i in range(B)])
    return out, res


def kernel(**inputs) -> np.ndarray:
    out, _ = _run(inputs, trace=False)
    return out


def bench(inputs):
    out, res = _run(inputs, trace=True)
    return out, res.exec_time_ns
